# revision 2
# baseline (speedup 1.0000x reference)
"""Trainium2 Bass kernel v3: hybrid sharding (batch-pair x H-half per core).

Core c (of 8): batch pair p = c%4 (batches 2p, 2p+1), h-half g = c//4
(h-tiles 8g .. 8g+7).  Each core computes partial outputs over its half of
H for its two batches; the host sums the two halves and adds b2.

Per core, loop over k = 0..7 h-tiles:
  DMA w1 tile (split across sync HW queue + gpsimd SW queue)
  matmul1: psum1[128h, 288] = W1tile^T @ [trig_b0|trig_b1|arg_b0|arg_b1]
  ACT evacs (fp16): tb4[128, 192] = trig both batches dup-paired + b1;
                    argh2[128, 192] = arg both batches
  per batch b in (0,1):
    DVE TT add 2x: pre[128, 4608] = argh bcast + tb bcast (dup-pair APs)
    relu: DVE tensor_scalar(max) 4x or ACT Relu (split)
    matmul2: 9 chunk MMs N=512, column-tiled, accumulating over k into
             psum chunk slot (b*9+c): bank //4, quadrant %4
  end: evac 18 psum slots -> out_sb, DMA out [2, 2, 4608] f32.
"""

import sys

if "/opt/trn_rl_repo" not in sys.path:
    sys.path.insert(0, "/opt/trn_rl_repo")

import numpy as np

B, T, E, D, H, O = 8, 48, 96, 768, 2048, 2
HT = H // 128
HTC = HT // 2            # 8 h-tiles per core
DT2 = (2 * D) // 128     # 12
DT = D // 128            # 6
TE = T * E               # 4608
CH = 512
NCH = TE // CH           # 9
NB = 2                   # batches per core
# iteration index i = k*2 + b (0..15): which do relu on ACT
ACT_RELU_ITERS = frozenset([1, 2, 4, 5, 6, 8, 9, 11, 12, 13])

_cache = {}


def _split_excess_waits(nc, mybir, max_waits=1):
    n_split = 0
    for f in nc.m.functions:
        for bb in f.blocks:
            new_insts = []
            for ins in bb.instructions:
                si = getattr(ins, "sync_info", None)
                ow = list(si.on_wait) if (si and si.on_wait) else []
                if len(ow) > max_waits:
                    head, rest = ow[:-max_waits], ow[-max_waits:]
                    for k in range(0, len(head), max_waits):
                        nop = mybir.InstNoOp(
                            name=nc.get_next_instruction_name(), ins=[], outs=[]
                        )
                        nop.engine = ins.engine
                        nop.sync_info = mybir.SyncInfo(
                            on_wait=head[k : k + max_waits], on_update=[]
                        )
                        nop.bass_nofuse = True
                        new_insts.append(nop)
                        n_split += 1
                    si.on_wait = rest
                new_insts.append(ins)
            bb.instructions[:] = new_insts
    return n_split


def _build_nc():
    import concourse.bass as bass
    import concourse.mybir as mybir
    import concourse.tile as tile
    from contextlib import ExitStack

    dt = mybir.dt
    alu = mybir.AluOpType
    act_fn = mybir.ActivationFunctionType
    f16 = dt.float16
    XC = 2 * T + 2 * E   # 288 moving cols: [trig_b0|trig_b1|arg_b0|arg_b1]

    nc = bass.Bass()
    xt_d = nc.declare_dram_parameter("xt", [128, DT, XC], f16, isOutput=False)
    w1_d = nc.declare_dram_parameter("w1t", [HTC, 128, DT2, 128], f16, isOutput=False)
    w2_d = nc.declare_dram_parameter("w2t", [128, HTC, O], f16, isOutput=False)
    b1_d = nc.declare_dram_parameter("b1t", [128, HTC], dt.float32, isOutput=False)
    # out2: [quadrant, o, bank, 512] f32; chunk s=(b*9+c) lives at
    # quadrant s%4, bank s//4 -> host reassembles
    out_d = nc.declare_dram_parameter("out", [4, O, 5, CH], dt.float32, isOutput=True)

    with ExitStack() as ctx:
        tc = ctx.enter_context(tile.TileContext(nc))
        consts = ctx.enter_context(tc.tile_pool(name="consts", bufs=1))
        w1pool = ctx.enter_context(tc.tile_pool(name="w1pool", bufs=4))
        evpool = ctx.enter_context(tc.tile_pool(name="evpool", bufs=3))
        prepool = ctx.enter_context(tc.tile_pool(name="prepool", bufs=3))
        hidpool = ctx.enter_context(tc.tile_pool(name="hidpool", bufs=3))
        psA = ctx.enter_context(tc.tile_pool(name="psA", bufs=2, space="PSUM"))
        psO = ctx.enter_context(tc.tile_pool(name="psO", bufs=1, space="PSUM"))

        xt = consts.tile([128, DT, XC], f16)
        w2sb = consts.tile([128, HTC, O], f16)
        b1sb = consts.tile([128, HTC], dt.float32)
        out_sb = consts.tile([128, 5, CH], dt.float32)

        nc.sync.dma_start(xt[:, 0:3, :], xt_d[:, 0:3, :])
        nc.gpsimd.dma_start(xt[:, 3:6, :], xt_d[:, 3:6, :])
        nc.gpsimd.dma_start(w2sb[:], w2_d[:])
        nc.gpsimd.dma_start(b1sb[:], b1_d[:])

        # PE warmup: dummy matmuls (no data deps) to lift HAM to 8/8 while
        # the input DMAs are still in flight.
        warm_sb = consts.tile([128, 16], f16)
        nc.gpsimd.memset(warm_sb[:], 0.0)
        # preload ACT's table set before the first real evac needs it
        nc.scalar.activation(warm_sb[:, 8:16], warm_sb[:, 0:8], act_fn.Relu)

        # 18 chunk slots (b*9+c): bank s//4 (5 banks), quadrant s%4
        psum_out = [
            psO.tile([128, CH], dt.float32, name=f"psum_out{bk}") for bk in range(5)
        ]

        for k in range(HTC):
            w1k = w1pool.tile([128, DT2, 128], f16, tag="w1k")
            nc.sync.dma_start(w1k[:, 0:6, :], w1_d[k, :, 0:6, :])
            nc.gpsimd.dma_start(w1k[:, 6:12, :], w1_d[k, :, 6:12, :])

            psum1 = psA.tile([128, XC], dt.float32, tag="psum1")
            for j in range(DT):
                nc.tensor.matmul(
                    psum1[:, 0 : 2 * T], lhsT=w1k[:, j, :], rhs=xt[:, j, 0 : 2 * T],
                    start=(j == 0), stop=(j == DT - 1),
                )
            for j in range(DT):
                nc.tensor.matmul(
                    psum1[:, 2 * T : XC], lhsT=w1k[:, DT + j, :], rhs=xt[:, j, 2 * T : XC],
                    start=(j == 0), stop=(j == DT - 1),
                )

            # --- evacuations (ACT, both batches in one op) ---
            # tb4: [128, 2, 2T]: batch-major, value t at cols 2t, 2t+1, +b1
            tb4 = evpool.tile([128, NB, 2 * T], f16, tag="tb4")
            hp = tc.high_priority()
            hp.__enter__()
            for b in range(NB):
                tb4_out = tb4[:, b, :].rearrange("p (t pair) -> p t pair", pair=2)
                trig_src = (
                    psum1[:, b * T : (b + 1) * T]
                    .unsqueeze(2)
                    .to_broadcast([128, T, 2])
                )
                nc.scalar.activation(
                    tb4_out, trig_src, act_fn.Identity, bias=b1sb[:, k : k + 1]
                )
            argh2 = evpool.tile([128, NB, E], f16, tag="argh2")
            if k in (2, 4, 5, 6):
                nc.vector.tensor_copy(argh2[:], psum1[:, 2 * T : XC])
            else:
                nc.scalar.activation(argh2[:], psum1[:, 2 * T : XC], act_fn.Identity)
            hp.__exit__(None, None, None)

            pre2 = prepool.tile([128, NB, TE], f16, tag="pre2")
            for b in range(NB):
                pre_pv = pre2[:, b, :].rearrange(
                    "p (t e2 pair) -> p t e2 pair", t=T, pair=2
                )
                a_pv = (
                    argh2[:, b, :]
                    .rearrange("p (e2 pair) -> p e2 pair", pair=2)
                    .unsqueeze(1)
                    .to_broadcast([128, T, E // 2, 2])
                )
                tb_pv = (
                    tb4[:, b, :]
                    .rearrange("p (t pair) -> p t pair", pair=2)
                    .unsqueeze(2)
                    .to_broadcast([128, T, E // 2, 2])
                )
                nc.vector.tensor_tensor(pre_pv, a_pv, tb_pv, alu.add)

            hid2 = hidpool.tile([128, NB, TE], f16, tag="hid2")
            on_act = [(k * 2 + b) in ACT_RELU_ITERS for b in range(NB)]
            if not any(on_act):
                nc.vector.tensor_scalar(hid2[:], pre2[:], 0.0, None, alu.max)
            else:
                for b in range(NB):
                    if on_act[b]:
                        nc.scalar.activation(hid2[:, b, :], pre2[:, b, :], act_fn.Relu)
                    else:
                        nc.vector.tensor_scalar(
                            hid2[:, b, :], pre2[:, b, :], 0.0, None, alu.max
                        )

            for b in range(NB):
                for c in range(NCH):
                    s = b * NCH + c
                    q = s % 4
                    nc.tensor.matmul(
                        psum_out[s // 4][32 * q : 32 * q + O, :],
                        lhsT=w2sb[:, k, :],
                        rhs=hid2[:, b, c * CH : (c + 1) * CH],
                        start=(k == 0), stop=(k == HTC - 1),
                        tile_position=(0, 32 * q),
                    )

        for bk in range(5):
            if bk % 2 == 0:
                nc.vector.tensor_copy(out_sb[:, bk, :], psum_out[bk][:])
            else:
                nc.scalar.activation(out_sb[:, bk, :], psum_out[bk][:], act_fn.Identity)
        for q in range(4):
            eng = nc.sync if q % 2 == 0 else nc.scalar
            eng.dma_start(out_d[q], out_sb[32 * q : 32 * q + O, :, :])

    _split_excess_waits(nc, mybir)
    return nc


def _prep_inputs(trig_embed, arg_embed, W1, b1, W2, b2):
    f16 = np.float16
    # per h-half weights
    w1t_full = np.ascontiguousarray(
        W1.reshape(DT2, 128, HT, 128).transpose(2, 1, 0, 3)
    ).astype(f16)                                   # [HT, 128, DT2, 128]
    w2t_full = np.ascontiguousarray(
        W2.reshape(HT, 128, O).transpose(1, 0, 2)
    ).astype(f16)                                   # [128, HT, O]
    b1t_full = np.ascontiguousarray(b1.reshape(HT, 128).T).astype(np.float32)
    in_maps = []
    for c in range(B):
        p, g = c % 4, c // 4
        b0, b1i = 2 * p, 2 * p + 1
        xTb = np.concatenate(
            [trig_embed[b0].T, trig_embed[b1i].T, arg_embed[b0].T, arg_embed[b1i].T],
            axis=1,
        )                                           # [D, 288]
        xt = np.ascontiguousarray(
            xTb.reshape(DT, 128, 2 * T + 2 * E).transpose(1, 0, 2)
        ).astype(f16)
        sl = slice(8 * g, 8 * g + 8)
        in_maps.append(
            {
                "xt": xt,
                "w1t": np.ascontiguousarray(w1t_full[sl]),
                "w2t": np.ascontiguousarray(w2t_full[:, sl, :]),
                "b1t": np.ascontiguousarray(b1t_full[:, sl]),
            }
        )
    return in_maps


def run(inputs, trace=False):
    from concourse.bass_utils import run_bass_kernel_spmd

    if "nc" not in _cache:
        _cache["nc"] = _build_nc()
    nc = _cache["nc"]
    in_maps = _prep_inputs(**inputs)
    res = run_bass_kernel_spmd(nc, in_maps, core_ids=list(range(B)), trace=trace)
    outs = np.stack([res.results[c]["out"] for c in range(B)])  # [8, 4, O, 5, CH]
    # chunk s = b*9+c at [quadrant s%4, :, bank s//4, :]
    per_core = np.empty((B, NB, O, TE), dtype=np.float32)
    for cc in range(B):
        for b in range(NB):
            for c in range(NCH):
                sidx = b * NCH + c
                per_core[cc, b, :, c * CH : (c + 1) * CH] = outs[
                    cc, sidx % 4, :, sidx // 4, :
                ]
    full = np.empty((B, T, E, O), dtype=np.float32)
    for p in range(4):
        for j in range(NB):
            s = per_core[p, j] + per_core[p + 4, j]  # [O, TE]
            full[2 * p + j] = s.T.reshape(T, E, O)
    full += inputs["b2"].reshape(1, 1, 1, O).astype(np.float32)
    return full, res


def kernel(**inputs):
    full, _ = run(inputs, trace=False)
    return full
